# revision 7
# baseline (speedup 1.0000x reference)
"""Memristive fully-connected layer on 8 Trainium2 NeuronCores.

The reference's differential conductance pair collapses algebraically:
g_pos - g_neg = g_eff = k_cond * weights, and the final rescale divides
K_V * k_cond back out, so the module computes exactly y = x @ w + b.

Strategy: data-parallel over the batch. Each core computes a
(1024 x 4096) @ (4096 x 4096) + b GEMM slice. x and w are converted to
bf16 on the host (error ~2e-3, well inside the 2e-2 gate), which halves
HBM traffic and SBUF footprint and enables the PE's fast-weight-load
path, so LDWEIGHTS (~95 ns) hides fully under each 512-column matmul
(~216 ns). The x shard is pre-transposed on host so stationary-operand
tiles are contiguous; the whole xT shard (8.4 MB) stays resident in
SBUF and w streams from HBM exactly once per core. Bias is broadcast
across partitions once and added in fp32 on PSUM eviction by the
vector engine; the output stays fp32.

Per core: 8 n-blocks of 512 columns. Blocks 0-6 run the contraction in
k-groups (single k-tiles for the first 8 so the first matmul's data
lands ~2.5 us after DMA start, then 4-tile batches), sweeping all 8
output-row tiles per group so PSUM evictions never stall the PE. The
final block runs k-contiguous per output-row tile against a fully
pre-staged w column block, so its evictions and output stores pipeline
during the block instead of trailing it. DMAs are batched to respect
the 8 hardware DGE queues, with w on the SP queue and xT/outputs on
the Activation queue. A short burst of throwaway matmuls during the
initial DMA fill lifts the PE's HAM clock gate before real work
arrives.
"""

import numpy as np

import concourse.bass as bass  # noqa: F401  (registers engine classes)
import concourse.mybir as mybir
from concourse import bacc, tile
from concourse.bass_utils import run_bass_kernel_spmd

dt = mybir.dt

BATCH, N_IN, N_OUT = 8192, 4096, 4096
NCORES = 8
MB = BATCH // NCORES          # 1024 batch rows per core
P = 128
KT = N_IN // P                # 32 contraction tiles
MT = MB // P                  # 8 output-row tiles per core
NBLK = 512                    # matmul free dim (one PSUM bank)
NB = N_OUT // NBLK            # 8 output-column blocks
KB = 4                        # k-tiles per k-block (per w DMA)
NKB = KT // KB                # 8 k-blocks
WARMUP_MM = 22

_cache = {}


def _build():
    nc = bacc.Bacc("TRN2", target_bir_lowering=False, debug=False)
    xT = nc.dram_tensor("xT", [N_IN, MB], dt.bfloat16, kind="ExternalInput")
    w = nc.dram_tensor("w", [N_IN, N_OUT], dt.bfloat16, kind="ExternalInput")
    b = nc.dram_tensor("b", [1, N_OUT], dt.float32, kind="ExternalInput")
    y = nc.dram_tensor("y", [MB, N_OUT], dt.float32, kind="ExternalOutput")

    xT_r = xT.rearrange("(kt p) m -> p kt m", p=P)    # [128, 32, 1024]
    w_r = w.rearrange("(kt p) n -> p kt n", p=P)      # [128, 32, 4096]
    y_r = y.rearrange("(mt p) n -> p mt n", p=P)      # [128, 8, 4096]

    with tile.TileContext(nc) as tc:
        with (
            tc.tile_pool(name="xtp", bufs=1) as xtp,
            tc.tile_pool(name="wp", bufs=6) as wp,
            tc.tile_pool(name="w7p", bufs=1) as w7p,
            tc.tile_pool(name="bp", bufs=1) as bp,
            tc.tile_pool(name="op", bufs=3) as op,
            tc.tile_pool(name="ps", bufs=1, space="PSUM") as ps,
        ):
            # w k-block DMA, 4 k-tiles per transfer on the SP queue.
            # Returns the block as a list of per-k-tile [128, 512] views.
            def w_dma(nb, kb):
                wt = wp.tile([P, KB, NBLK], dt.bfloat16, name="wt")
                nc.sync.dma_start(
                    wt[:],
                    w_r[:, kb * KB:(kb + 1) * KB, nb * NBLK:(nb + 1) * NBLK],
                )
                return [wt[:, kk, :] for kk in range(KB)]

            # single-k-tile w DMA (startup: minimal first-matmul latency)
            def w_dma1(nb, kt):
                wt = wp.tile([P, 1, NBLK], dt.bfloat16, name="wt1")
                nc.sync.dma_start(
                    wt[:],
                    w_r[:, kt:kt + 1, nb * NBLK:(nb + 1) * NBLK],
                )
                return [wt[:, 0, :]]

            xts = xtp.tile([P, KT, MB], dt.bfloat16, name="xts")

            def xt_dma(kb):
                nc.scalar.dma_start(
                    xts[:, kb * KB:(kb + 1) * KB, :],
                    xT_r[:, kb * KB:(kb + 1) * KB, :],
                )

            def xt_dma1(kt, halves=False):
                if halves:
                    # two transfers so the first m-tiles' data lands sooner
                    nc.scalar.dma_start(
                        xts[:, kt:kt + 1, :MB // 2],
                        xT_r[:, kt:kt + 1, :MB // 2],
                    )
                    nc.scalar.dma_start(
                        xts[:, kt:kt + 1, MB // 2:],
                        xT_r[:, kt:kt + 1, MB // 2:],
                    )
                else:
                    nc.scalar.dma_start(
                        xts[:, kt:kt + 1, :],
                        xT_r[:, kt:kt + 1, :],
                    )

            # HAM warmup: throwaway matmuls on a zeroed tile while the
            # first DMAs are in flight, so real matmuls start at 2.4 GHz.
            warm = bp.tile([P, 256], dt.bfloat16, name="warm")
            nc.gpsimd.memset(warm[:], 0.0)
            wpsums = [
                ps.tile([P, NBLK], dt.float32, name=f"ps{i}") for i in range(MT)
            ]
            for i in range(WARMUP_MM):
                nc.tensor.matmul(
                    wpsums[i % MT][:, :256], warm[:, :P], warm[:],
                    start=True, stop=True,
                )

            # Startup DMAs in consumption order, alternating the two HWDGE
            # rings (w on SP, xT on ACT) at single-k-tile granularity for
            # the first 8 k-tiles so the first matmul's 0.4 MB of data
            # lands ~2.5 us after DMA start instead of ~9 us.
            nb0_w = {}
            for kt in range(2 * KB):
                nb0_w[kt] = w_dma1(0, kt)[0]
                xt_dma1(kt, halves=(kt < 2))
            nb0_wb = {}
            for kb in range(2, NKB):
                nb0_wb[kb] = w_dma(0, kb)
                xt_dma(kb)

            # Bias: DMA the row into partition 0 of bias_sb, then broadcast
            # in place. Emitted after the startup DMAs — it rides the slow
            # gpsimd queue and is only needed at the first eviction.
            bias_sb = bp.tile([P, N_OUT], dt.float32, name="bias_sb")
            nc.scalar.dma_start(bias_sb[0:1, :], b[:, :])
            nc.gpsimd.partition_broadcast(bias_sb[:], bias_sb[0:1, :])

            # Final n-block's w column: fully staged ahead of time so the
            # last block can run k-contiguous per m-tile. 4 transfers of
            # 8 k-tiles (0.5 MB), emitted interleaved into nb=5's stream.
            w7 = w7p.tile([P, KT, NBLK], dt.bfloat16, name="w7t")

            def w7_dma(i):
                nc.sync.dma_start(
                    w7[:, i * 8:(i + 1) * 8, :],
                    w_r[:, i * 8:(i + 1) * 8, (NB - 1) * NBLK:NB * NBLK],
                )

            # k-group schedule: nb=0 starts with single k-tiles.
            groups0 = [[kt] for kt in range(2 * KB)] + [
                list(range(kb * KB, (kb + 1) * KB)) for kb in range(2, NKB)
            ]
            groups = [list(range(kb * KB, (kb + 1) * KB)) for kb in range(NKB)]

            for nb in range(NB - 1):
                psums = [
                    ps.tile([P, NBLK], dt.float32, name=f"ps{m}")
                    for m in range(MT)
                ]
                ot = None
                gs = groups0 if nb == 0 else groups
                for gi, ks in enumerate(gs):
                    if nb == 0:
                        wts = ([nb0_w[ks[0]]] if len(ks) == 1
                               else nb0_wb[ks[0] // KB])
                    else:
                        wts = w_dma(nb, ks[0] // KB)
                        if nb == 5 and gi % 2 == 1:
                            w7_dma(gi // 2)
                    last_group = gi == len(gs) - 1
                    for m in range(MT):
                        for kk, k in enumerate(ks):
                            nc.tensor.matmul(
                                psums[m][:],
                                xts[:, k, m * P:(m + 1) * P],
                                wts[kk],
                                start=(k == 0),
                                stop=(k == KT - 1),
                            )
                        if last_group:
                            if m % 2 == 0:
                                ot = op.tile([P, 2, NBLK], dt.float32, name="ot")
                            nc.vector.tensor_add(
                                ot[:, m % 2, :],
                                psums[m][:],
                                bias_sb[:, nb * NBLK:(nb + 1) * NBLK],
                            )
                            if m % 2 == 1:
                                nc.scalar.dma_start(
                                    y_r[:, m - 1:m + 1, nb * NBLK:(nb + 1) * NBLK],
                                    ot[:],
                                )

            # Final n-block: k-contiguous per m-tile against resident w7.
            # Each m-tile's 32-matmul accumulation completes ~7 us apart,
            # so the bias-add + store of m pipelines under m+1's matmuls
            # and only the last m's eviction trails the final matmul.
            nb = NB - 1
            for m in range(MT):
                psum = ps.tile([P, NBLK], dt.float32, name=f"ps{m}")
                for k in range(KT):
                    nc.tensor.matmul(
                        psum[:],
                        xts[:, k, m * P:(m + 1) * P],
                        w7[:, k, :],
                        start=(k == 0),
                        stop=(k == KT - 1),
                    )
                ot = op.tile([P, 1, NBLK], dt.float32, name="ot1")
                if m == MT - 1:
                    # last eviction is the kernel tail: halve it and run
                    # both stores in parallel on the two HWDGE queues
                    h = NBLK // 2
                    for hi, eng in enumerate((nc.sync, nc.scalar)):
                        nc.vector.tensor_add(
                            ot[:, 0, hi * h:(hi + 1) * h],
                            psum[:, hi * h:(hi + 1) * h],
                            bias_sb[:, nb * NBLK + hi * h:nb * NBLK + (hi + 1) * h],
                        )
                        eng.dma_start(
                            y_r[:, m:m + 1, nb * NBLK + hi * h:nb * NBLK + (hi + 1) * h],
                            ot[:, :, hi * h:(hi + 1) * h],
                        )
                else:
                    nc.vector.tensor_add(
                        ot[:, 0, :],
                        psum[:],
                        bias_sb[:, nb * NBLK:(nb + 1) * NBLK],
                    )
                    eng = nc.scalar if m % 2 else nc.sync
                    eng.dma_start(
                        y_r[:, m:m + 1, nb * NBLK:(nb + 1) * NBLK],
                        ot[:],
                    )
    nc.compile()
    return nc


def kernel(x, w, b, _trace=False, _trace_kwargs=None):
    import ml_dtypes

    if "nc" not in _cache:
        _cache["nc"] = _build()
    nc = _cache["nc"]

    bf16 = ml_dtypes.bfloat16
    b2 = np.ascontiguousarray(np.asarray(b, dtype=np.float32).reshape(1, N_OUT))
    w2 = np.ascontiguousarray(np.asarray(w, dtype=np.float32).astype(bf16))
    xT_all = np.asarray(x, dtype=np.float32).T.astype(bf16)  # [N_IN, BATCH]
    in_maps = []
    for c in range(NCORES):
        xs = np.ascontiguousarray(xT_all[:, c * MB:(c + 1) * MB])
        in_maps.append({"xT": xs, "w": w2, "b": b2})

    res = run_bass_kernel_spmd(
        nc,
        in_maps,
        core_ids=list(range(NCORES)),
        trace=_trace,
        **(_trace_kwargs or {}),
    )
    out = np.concatenate([res.results[c]["y"] for c in range(NCORES)], axis=0)
    if _trace:
        return out, res
    return out


# revision 11
# speedup vs baseline: 1.0067x; 1.0067x over previous
"""Memristive fully-connected layer on 8 Trainium2 NeuronCores.

The reference's differential conductance pair collapses algebraically:
g_pos - g_neg = g_eff = k_cond * weights, and the final rescale divides
K_V * k_cond back out, so the module computes exactly y = x @ w + b.

Strategy: data-parallel over the batch. Each core computes a
(1024 x 4096) @ (4096 x 4096) + b GEMM slice. x and w are converted to
bf16 on the host (error ~2e-3, well inside the 2e-2 gate), which halves
HBM traffic and SBUF footprint and enables the PE's fast-weight-load
path, so LDWEIGHTS (~95 ns) hides fully under each 512-column matmul
(~216 ns). The x shard is pre-transposed on host so stationary-operand
tiles are contiguous; the whole xT shard (8.4 MB) stays resident in
SBUF and w streams from HBM exactly once per core. Bias is broadcast
across partitions once and added in fp32 on PSUM eviction by the
vector engine; the output stays fp32.

Per core: 8 n-blocks of 512 columns. Blocks 0-6 run the contraction in
k-groups (single k-tiles for the first 8 so the first matmul's data
lands ~2.5 us after DMA start, then 4-tile batches), sweeping all 8
output-row tiles per group so PSUM evictions never stall the PE. The
final block runs k-contiguous per output-row tile against a fully
pre-staged w column block, so its evictions and output stores pipeline
during the block instead of trailing it. DMAs are batched to respect
the 8 hardware DGE queues, with w on the SP queue and xT/outputs on
the Activation queue. A short burst of throwaway matmuls during the
initial DMA fill lifts the PE's HAM clock gate before real work
arrives.
"""

import numpy as np

import concourse.bass as bass  # noqa: F401  (registers engine classes)
import concourse.mybir as mybir
from concourse import bacc, tile
from concourse.bass_utils import run_bass_kernel_spmd

dt = mybir.dt

BATCH, N_IN, N_OUT = 8192, 4096, 4096
NCORES = 8
MB = BATCH // NCORES          # 1024 batch rows per core
P = 128
KT = N_IN // P                # 32 contraction tiles
MT = MB // P                  # 8 output-row tiles per core
NBLK = 512                    # matmul free dim (one PSUM bank)
NB = N_OUT // NBLK            # 8 output-column blocks
KB = 4                        # k-tiles per k-block (per w DMA)
NKB = KT // KB                # 8 k-blocks
WARMUP_MM = 18

_cache = {}


def _build():
    nc = bacc.Bacc("TRN2", target_bir_lowering=False, debug=False)
    xT = nc.dram_tensor("xT", [N_IN, MB], dt.bfloat16, kind="ExternalInput")
    w = nc.dram_tensor("w", [N_IN, N_OUT], dt.bfloat16, kind="ExternalInput")
    b = nc.dram_tensor("b", [1, N_OUT], dt.float32, kind="ExternalInput")
    y = nc.dram_tensor("y", [MB, N_OUT], dt.float32, kind="ExternalOutput")

    xT_r = xT.rearrange("(kt p) m -> p kt m", p=P)    # [128, 32, 1024]
    w_r = w.rearrange("(kt p) n -> p kt n", p=P)      # [128, 32, 4096]
    y_r = y.rearrange("(mt p) n -> p mt n", p=P)      # [128, 8, 4096]

    with tile.TileContext(nc) as tc:
        with (
            tc.tile_pool(name="xtp", bufs=1) as xtp,
            tc.tile_pool(name="wp", bufs=6) as wp,
            tc.tile_pool(name="w7p", bufs=1) as w7p,
            tc.tile_pool(name="bp", bufs=1) as bp,
            tc.tile_pool(name="op", bufs=3) as op,
            tc.tile_pool(name="ps", bufs=1, space="PSUM") as ps,
        ):
            # w k-block DMA, 4 k-tiles per transfer on the SP queue.
            # Returns the block as a list of per-k-tile [128, 512] views.
            def w_dma(nb, kb):
                wt = wp.tile([P, KB, NBLK], dt.bfloat16, name="wt")
                nc.sync.dma_start(
                    wt[:],
                    w_r[:, kb * KB:(kb + 1) * KB, nb * NBLK:(nb + 1) * NBLK],
                )
                return [wt[:, kk, :] for kk in range(KB)]

            # variable-size w group DMA (startup ramp)
            def w_dma_g(nb, ks):
                wt = wp.tile([P, len(ks), NBLK], dt.bfloat16,
                             name=f"wt{len(ks)}")
                nc.sync.dma_start(
                    wt[:],
                    w_r[:, ks[0]:ks[0] + len(ks), nb * NBLK:(nb + 1) * NBLK],
                )
                return [wt[:, i, :] for i in range(len(ks))]

            xts = xtp.tile([P, KT, MB], dt.bfloat16, name="xts")

            def xt_dma_g(ks):
                nc.scalar.dma_start(
                    xts[:, ks[0]:ks[0] + len(ks), :],
                    xT_r[:, ks[0]:ks[0] + len(ks), :],
                )

            # HAM warmup: throwaway matmuls on a zeroed tile while the
            # first DMAs are in flight, so real matmuls start at 2.4 GHz.
            warm = bp.tile([P, 256], dt.bfloat16, name="warm")
            nc.gpsimd.memset(warm[:], 0.0)
            wpsums = [
                ps.tile([P, NBLK], dt.float32, name=f"ps{i}") for i in range(MT)
            ]
            for i in range(WARMUP_MM):
                nc.tensor.matmul(
                    wpsums[i % MT][:, :256], warm[:, :P], warm[:],
                    start=True, stop=True,
                )

            # Startup DMAs in consumption order, alternating the two HWDGE
            # rings (w on SP, xT on ACT) with a size ramp (1,1,2,4 k-tiles)
            # so the first matmul's 0.4 MB of data lands ~2.5 us after DMA
            # start while descriptor generation (~0.8 us per transfer per
            # sequencer) stays off the critical path.
            nb0_groups = [[0], [1], [2, 3], [4, 5, 6, 7]] + [
                list(range(kb * KB, (kb + 1) * KB)) for kb in range(2, NKB)
            ]
            nb0_wts = []
            for ks in nb0_groups:
                nb0_wts.append(w_dma_g(0, ks))
                xt_dma_g(ks)

            # Bias: DMA the row into partition 0 of bias_sb, then broadcast
            # in place. Emitted after the startup DMAs — it rides the slow
            # gpsimd queue and is only needed at the first eviction.
            bias_sb = bp.tile([P, N_OUT], dt.float32, name="bias_sb")
            nc.scalar.dma_start(bias_sb[0:1, :], b[:, :])
            nc.gpsimd.partition_broadcast(bias_sb[:], bias_sb[0:1, :])

            # Final n-block's w column: fully staged ahead of time so the
            # last block can run k-contiguous per m-tile. 4 transfers of
            # 8 k-tiles (0.5 MB), emitted interleaved into nb=5's stream.
            w7 = w7p.tile([P, KT, NBLK], dt.bfloat16, name="w7t")

            def w7_dma(i):
                nc.sync.dma_start(
                    w7[:, i * 8:(i + 1) * 8, :],
                    w_r[:, i * 8:(i + 1) * 8, (NB - 1) * NBLK:NB * NBLK],
                )

            # k-group schedule: nb=0 uses the startup ramp groups.
            groups = [list(range(kb * KB, (kb + 1) * KB)) for kb in range(NKB)]

            for nb in range(NB - 1):
                psums = [
                    ps.tile([P, NBLK], dt.float32, name=f"ps{m}")
                    for m in range(MT)
                ]
                ot = None
                gs = nb0_groups if nb == 0 else groups
                for gi, ks in enumerate(gs):
                    if nb == 0:
                        wts = nb0_wts[gi]
                    else:
                        wts = w_dma(nb, ks[0] // KB)
                        if nb == 5 and gi % 2 == 1:
                            w7_dma(gi // 2)
                    last_group = gi == len(gs) - 1
                    for m in range(MT):
                        for kk, k in enumerate(ks):
                            nc.tensor.matmul(
                                psums[m][:],
                                xts[:, k, m * P:(m + 1) * P],
                                wts[kk],
                                start=(k == 0),
                                stop=(k == KT - 1),
                            )
                        if last_group:
                            if m % 2 == 0:
                                ot = op.tile([P, 2, NBLK], dt.float32, name="ot")
                            nc.vector.tensor_add(
                                ot[:, m % 2, :],
                                psums[m][:],
                                bias_sb[:, nb * NBLK:(nb + 1) * NBLK],
                            )
                            if m % 2 == 1:
                                nc.scalar.dma_start(
                                    y_r[:, m - 1:m + 1, nb * NBLK:(nb + 1) * NBLK],
                                    ot[:],
                                )

            # Final n-block: k-contiguous per m-tile against resident w7.
            # Each m-tile's 32-matmul accumulation completes ~7 us apart,
            # so the bias-add + store of m pipelines under m+1's matmuls
            # and only the last m's eviction trails the final matmul.
            nb = NB - 1
            for m in range(MT):
                psum = ps.tile([P, NBLK], dt.float32, name=f"ps{m}")
                for k in range(KT):
                    nc.tensor.matmul(
                        psum[:],
                        xts[:, k, m * P:(m + 1) * P],
                        w7[:, k, :],
                        start=(k == 0),
                        stop=(k == KT - 1),
                    )
                ot = op.tile([P, 1, NBLK], dt.float32, name="ot1")
                if m == MT - 1:
                    # last eviction is the kernel tail: halve it and run
                    # both stores in parallel on the two HWDGE queues
                    h = NBLK // 2
                    for hi, eng in enumerate((nc.sync, nc.scalar)):
                        nc.vector.tensor_add(
                            ot[:, 0, hi * h:(hi + 1) * h],
                            psum[:, hi * h:(hi + 1) * h],
                            bias_sb[:, nb * NBLK + hi * h:nb * NBLK + (hi + 1) * h],
                        )
                        eng.dma_start(
                            y_r[:, m:m + 1, nb * NBLK + hi * h:nb * NBLK + (hi + 1) * h],
                            ot[:, :, hi * h:(hi + 1) * h],
                        )
                else:
                    nc.vector.tensor_add(
                        ot[:, 0, :],
                        psum[:],
                        bias_sb[:, nb * NBLK:(nb + 1) * NBLK],
                    )
                    eng = nc.scalar if m % 2 else nc.sync
                    eng.dma_start(
                        y_r[:, m:m + 1, nb * NBLK:(nb + 1) * NBLK],
                        ot[:],
                    )
    nc.compile()
    return nc


def kernel(x, w, b, _trace=False, _trace_kwargs=None):
    import ml_dtypes

    if "nc" not in _cache:
        _cache["nc"] = _build()
    nc = _cache["nc"]

    bf16 = ml_dtypes.bfloat16
    b2 = np.ascontiguousarray(np.asarray(b, dtype=np.float32).reshape(1, N_OUT))
    w2 = np.ascontiguousarray(np.asarray(w, dtype=np.float32).astype(bf16))
    xT_all = np.asarray(x, dtype=np.float32).T.astype(bf16)  # [N_IN, BATCH]
    in_maps = []
    for c in range(NCORES):
        xs = np.ascontiguousarray(xT_all[:, c * MB:(c + 1) * MB])
        in_maps.append({"xT": xs, "w": w2, "b": b2})

    res = run_bass_kernel_spmd(
        nc,
        in_maps,
        core_ids=list(range(NCORES)),
        trace=_trace,
        **(_trace_kwargs or {}),
    )
    out = np.concatenate([res.results[c]["y"] for c in range(NCORES)], axis=0)
    if _trace:
        return out, res
    return out


# revision 13
# speedup vs baseline: 1.0070x; 1.0003x over previous
"""Memristive fully-connected layer on 8 Trainium2 NeuronCores.

The reference's differential conductance pair collapses algebraically:
g_pos - g_neg = g_eff = k_cond * weights, and the final rescale divides
K_V * k_cond back out, so the module computes exactly y = x @ w + b.

Strategy: data-parallel over the batch. Each core computes a
(1024 x 4096) @ (4096 x 4096) + b GEMM slice. x and w are converted to
bf16 on the host (error ~2e-3, well inside the 2e-2 gate), which halves
HBM traffic and SBUF footprint and enables the PE's fast-weight-load
path, so LDWEIGHTS (~95 ns) hides fully under each 512-column matmul
(~216 ns). The x shard is pre-transposed on host so stationary-operand
tiles are contiguous; the whole xT shard (8.4 MB) stays resident in
SBUF and w streams from HBM exactly once per core. Bias is broadcast
across partitions once and added in fp32 on PSUM eviction by the
vector engine; the output stays fp32.

Per core: 8 n-blocks of 512 columns. Blocks 0-6 run the contraction in
k-groups (single k-tiles for the first 8 so the first matmul's data
lands ~2.5 us after DMA start, then 4-tile batches), sweeping all 8
output-row tiles per group so PSUM evictions never stall the PE. The
final block runs k-contiguous per output-row tile against a fully
pre-staged w column block, so its evictions and output stores pipeline
during the block instead of trailing it. DMAs are batched to respect
the 8 hardware DGE queues, with w on the SP queue and xT/outputs on
the Activation queue. A short burst of throwaway matmuls during the
initial DMA fill lifts the PE's HAM clock gate before real work
arrives.
"""

import numpy as np

import concourse.bass as bass  # noqa: F401  (registers engine classes)
import concourse.mybir as mybir
from concourse import bacc, tile
from concourse.bass_utils import run_bass_kernel_spmd

dt = mybir.dt

BATCH, N_IN, N_OUT = 8192, 4096, 4096
NCORES = 8
MB = BATCH // NCORES          # 1024 batch rows per core
P = 128
KT = N_IN // P                # 32 contraction tiles
MT = MB // P                  # 8 output-row tiles per core
NBLK = 512                    # matmul free dim (one PSUM bank)
NB = N_OUT // NBLK            # 8 output-column blocks
KB = 4                        # k-tiles per k-block (per w DMA)
NKB = KT // KB                # 8 k-blocks
WARMUP_MM = 110            # N=64 each: fine-grained PE-busy bridge (~4.7 us)

_cache = {}


def _build():
    nc = bacc.Bacc("TRN2", target_bir_lowering=False, debug=False)
    xT = nc.dram_tensor("xT", [N_IN, MB], dt.bfloat16, kind="ExternalInput")
    w = nc.dram_tensor("w", [N_IN, N_OUT], dt.bfloat16, kind="ExternalInput")
    b = nc.dram_tensor("b", [1, N_OUT], dt.float32, kind="ExternalInput")
    y = nc.dram_tensor("y", [MB, N_OUT], dt.float32, kind="ExternalOutput")

    xT_r = xT.rearrange("(kt p) m -> p kt m", p=P)    # [128, 32, 1024]
    w_r = w.rearrange("(kt p) n -> p kt n", p=P)      # [128, 32, 4096]
    y_r = y.rearrange("(mt p) n -> p mt n", p=P)      # [128, 8, 4096]

    with tile.TileContext(nc) as tc:
        with (
            tc.tile_pool(name="xtp", bufs=1) as xtp,
            tc.tile_pool(name="wp", bufs=6) as wp,
            tc.tile_pool(name="w7p", bufs=1) as w7p,
            tc.tile_pool(name="bp", bufs=1) as bp,
            tc.tile_pool(name="op", bufs=3) as op,
            tc.tile_pool(name="ps", bufs=1, space="PSUM") as ps,
        ):
            # w k-block DMA, 4 k-tiles per transfer on the SP queue.
            # Returns the block as a list of per-k-tile [128, 512] views.
            def w_dma(nb, kb):
                wt = wp.tile([P, KB, NBLK], dt.bfloat16, name="wt")
                nc.sync.dma_start(
                    wt[:],
                    w_r[:, kb * KB:(kb + 1) * KB, nb * NBLK:(nb + 1) * NBLK],
                )
                return [wt[:, kk, :] for kk in range(KB)]

            # variable-size w group DMA (startup ramp)
            def w_dma_g(nb, ks):
                wt = wp.tile([P, len(ks), NBLK], dt.bfloat16,
                             name=f"wt{len(ks)}")
                nc.sync.dma_start(
                    wt[:],
                    w_r[:, ks[0]:ks[0] + len(ks), nb * NBLK:(nb + 1) * NBLK],
                )
                return [wt[:, i, :] for i in range(len(ks))]

            xts = xtp.tile([P, KT, MB], dt.bfloat16, name="xts")

            def xt_dma_g(ks):
                nc.scalar.dma_start(
                    xts[:, ks[0]:ks[0] + len(ks), :],
                    xT_r[:, ks[0]:ks[0] + len(ks), :],
                )

            # HAM warmup: throwaway matmuls on a zeroed tile while the
            # first DMAs are in flight, so real matmuls start at 2.4 GHz.
            warm = bp.tile([P, P], dt.bfloat16, name="warm")
            nc.gpsimd.memset(warm[:], 0.0)
            wpsums = [
                ps.tile([P, NBLK], dt.float32, name=f"ps{i}") for i in range(MT)
            ]
            for i in range(WARMUP_MM):
                nc.tensor.matmul(
                    wpsums[i % MT][:, :64], warm[:, :P], warm[:, :64],
                    start=True, stop=True,
                )

            # Startup DMAs in consumption order, alternating the two HWDGE
            # rings (w on SP, xT on ACT) with a size ramp (1,1,2,4 k-tiles)
            # so the first matmul's 0.4 MB of data lands ~2.5 us after DMA
            # start while descriptor generation (~0.8 us per transfer per
            # sequencer) stays off the critical path.
            nb0_groups = [[0], [1], [2, 3], [4, 5, 6, 7]] + [
                list(range(kb * KB, (kb + 1) * KB)) for kb in range(2, NKB)
            ]
            nb0_wts = []
            for ks in nb0_groups:
                nb0_wts.append(w_dma_g(0, ks))
                xt_dma_g(ks)

            # Bias: DMA the row into partition 0 of bias_sb, then broadcast
            # in place. Emitted after the startup DMAs — it rides the slow
            # gpsimd queue and is only needed at the first eviction.
            bias_sb = bp.tile([P, N_OUT], dt.float32, name="bias_sb")
            nc.scalar.dma_start(bias_sb[0:1, :], b[:, :])
            nc.gpsimd.partition_broadcast(bias_sb[:], bias_sb[0:1, :])

            # Final n-block's w column: fully staged ahead of time so the
            # last block can run k-contiguous per m-tile. 4 transfers of
            # 8 k-tiles (0.5 MB), emitted interleaved into nb=5's stream.
            w7 = w7p.tile([P, KT, NBLK], dt.bfloat16, name="w7t")

            def w7_dma(i):
                nc.sync.dma_start(
                    w7[:, i * 8:(i + 1) * 8, :],
                    w_r[:, i * 8:(i + 1) * 8, (NB - 1) * NBLK:NB * NBLK],
                )

            # k-group schedule: nb=0 uses the startup ramp groups.
            groups = [list(range(kb * KB, (kb + 1) * KB)) for kb in range(NKB)]

            for nb in range(NB - 1):
                psums = [
                    ps.tile([P, NBLK], dt.float32, name=f"ps{m}")
                    for m in range(MT)
                ]
                ot = None
                gs = nb0_groups if nb == 0 else groups
                for gi, ks in enumerate(gs):
                    if nb == 0:
                        wts = nb0_wts[gi]
                    else:
                        wts = w_dma(nb, ks[0] // KB)
                        if nb == 5 and gi % 2 == 1:
                            w7_dma(gi // 2)
                    last_group = gi == len(gs) - 1
                    for m in range(MT):
                        for kk, k in enumerate(ks):
                            nc.tensor.matmul(
                                psums[m][:],
                                xts[:, k, m * P:(m + 1) * P],
                                wts[kk],
                                start=(k == 0),
                                stop=(k == KT - 1),
                            )
                        if last_group:
                            if m % 2 == 0:
                                ot = op.tile([P, 2, NBLK], dt.float32, name="ot")
                            nc.vector.tensor_add(
                                ot[:, m % 2, :],
                                psums[m][:],
                                bias_sb[:, nb * NBLK:(nb + 1) * NBLK],
                            )
                            if m % 2 == 1:
                                nc.scalar.dma_start(
                                    y_r[:, m - 1:m + 1, nb * NBLK:(nb + 1) * NBLK],
                                    ot[:],
                                )

            # Final n-block: k-contiguous per m-tile against resident w7.
            # Each m-tile's 32-matmul accumulation completes ~7 us apart,
            # so the bias-add + store of m pipelines under m+1's matmuls
            # and only the last m's eviction trails the final matmul.
            nb = NB - 1
            for m in range(MT):
                psum = ps.tile([P, NBLK], dt.float32, name=f"ps{m}")
                for k in range(KT):
                    nc.tensor.matmul(
                        psum[:],
                        xts[:, k, m * P:(m + 1) * P],
                        w7[:, k, :],
                        start=(k == 0),
                        stop=(k == KT - 1),
                    )
                ot = op.tile([P, 1, NBLK], dt.float32, name="ot1")
                if m == MT - 1:
                    # last eviction is the kernel tail: halve it and run
                    # both stores in parallel on the two HWDGE queues
                    h = NBLK // 2
                    for hi, eng in enumerate((nc.sync, nc.scalar)):
                        nc.vector.tensor_add(
                            ot[:, 0, hi * h:(hi + 1) * h],
                            psum[:, hi * h:(hi + 1) * h],
                            bias_sb[:, nb * NBLK + hi * h:nb * NBLK + (hi + 1) * h],
                        )
                        eng.dma_start(
                            y_r[:, m:m + 1, nb * NBLK + hi * h:nb * NBLK + (hi + 1) * h],
                            ot[:, :, hi * h:(hi + 1) * h],
                        )
                else:
                    nc.vector.tensor_add(
                        ot[:, 0, :],
                        psum[:],
                        bias_sb[:, nb * NBLK:(nb + 1) * NBLK],
                    )
                    eng = nc.scalar if m % 2 else nc.sync
                    eng.dma_start(
                        y_r[:, m:m + 1, nb * NBLK:(nb + 1) * NBLK],
                        ot[:],
                    )
    nc.compile()
    return nc


def kernel(x, w, b, _trace=False, _trace_kwargs=None):
    import ml_dtypes

    if "nc" not in _cache:
        _cache["nc"] = _build()
    nc = _cache["nc"]

    bf16 = ml_dtypes.bfloat16
    b2 = np.ascontiguousarray(np.asarray(b, dtype=np.float32).reshape(1, N_OUT))
    w2 = np.ascontiguousarray(np.asarray(w, dtype=np.float32).astype(bf16))
    xT_all = np.asarray(x, dtype=np.float32).T.astype(bf16)  # [N_IN, BATCH]
    in_maps = []
    for c in range(NCORES):
        xs = np.ascontiguousarray(xT_all[:, c * MB:(c + 1) * MB])
        in_maps.append({"xT": xs, "w": w2, "b": b2})

    res = run_bass_kernel_spmd(
        nc,
        in_maps,
        core_ids=list(range(NCORES)),
        trace=_trace,
        **(_trace_kwargs or {}),
    )
    out = np.concatenate([res.results[c]["y"] for c in range(NCORES)], axis=0)
    if _trace:
        return out, res
    return out
